# revision 1
# baseline (speedup 1.0000x reference)
"""Trainium2 Bass kernel for MemoryEfficientAttnBlock3D.

Computes: y = x + conv1x1(attn(conv1x1_{q,k,v}(groupnorm(x))), wp, bp)
for x of shape (2, 64, 32, 32, 8)  (B=2, C=64, N=8192 tokens per batch).

Sharding: 8 cores = 2 batches x 4 query-chunks of 2048 tokens.  Each core
receives its batch's full token volume ROTATED so that its query chunk is
always tokens [0:2048] -- groupnorm statistics and softmax/AV reductions are
permutation-invariant over kv tokens, so all cores run an identical program.

Algebraic folds done on the host:
  - gamma folds into Wq/Wk/Wv columns; the attention scale 1/sqrt(C) into Wq.
  - k's additive constant (Wk@beta + bk) shifts every score of a softmax row
    equally -> dropped exactly (softmax shift invariance).
  - bq enters scores via an extra contraction row: q carries a constant ones
    row, k an extra output row ((Wk_eff^T @ bq_eff) @ xn).
  - the OUTPUT projection wp folds into the v weights: sum_c wp[o,c] v[c,n]
    = (wp@Wv_eff) @ xn[:,n], so the AV matmul accumulates wp@AV directly and
    the standalone P matmul disappears.
  - v's additive constant and bp are applied as a per-partition scalar in
    the final fused (t + bp) + x DVE op.
  - softmax denominators: v^T carries a ones column, so the AV matmul
    accumulates [wp@AV | rowsum]; the division is applied at the very end
    (column scaling commutes with everything linear).

Precision: all attention / projection matmuls run in fp16 (fp32 matmuls are
4x slower on the PE: LOW_HIGH weight split x half-rate fp32 streaming, and
fp32 streams do not lift the HAM clock gate).  Projection weights are sent
as fp16 hi/lo pairs and applied in two accumulating passes, which removes
the weight-rounding error; groupnorm, statistics, softmax scores (PSUM),
rowsums and the final combine stay fp32.  Measured end-to-end absmax vs the
fp32 reference: 4.4e-4 on outputs with |out|max 5.3 (8.4e-5 of scale; the
all-fp32 floor is 1.5e-5, a bf16-native implementation would be 3e-3).
"""

import numpy as np

import concourse.bass as bass
import concourse.tile as tile
from concourse import bacc, mybir

F32 = mybir.dt.float32
F16 = mybir.dt.float16
AF = mybir.ActivationFunctionType
OP = mybir.AluOpType

C = 64
GROUPS = 32
EPS = 1e-6

B_FULL = 2
H_FULL, W_FULL, D_FULL = 32, 32, 8
N_FULL = H_FULL * W_FULL * D_FULL  # 8192 kv tokens per batch
N_CORES = 8
Q_CHUNKS = 4
M_FULL = N_FULL // Q_CHUNKS  # 2048 q tokens per core

MB = 512        # q-token block (one PSUM bank of fp32)
NT = 128        # kv-token tile (matmul M / partition dim)
GSZ = 3         # n-tiles per exp group ([128, 1536] PSUM tile = 3 banks)
STAT_CHUNK = 1024
PCH = 512       # projection chunk (tokens)


def emit(tc, nc, n_tok, m_tok, xb_d, wqh_d, wql_d, wkh_d, wkl_d, wvhl_d,
         bpc_d, pair_d, expand_d, out_d):
    ntiles = n_tok // NT
    nch = max(1, n_tok // STAT_CHUNK)
    sch = n_tok // nch
    nchunks = n_tok // PCH
    xch = max(1, n_tok // 2048)   # xh (normalize) macro-chunks
    xsz = n_tok // xch
    cpx = xsz // PCH              # projection chunks per xh chunk

    with (
        tc.tile_pool(name="persist", bufs=1) as persist,
        tc.tile_pool(name="expS", bufs=4) as epool,
        tc.tile_pool(name="mtail", bufs=3) as mpool,
        tc.tile_pool(name="spsum", bufs=2, space="PSUM") as spool,
        tc.tile_pool(name="avpsum", bufs=1, space="PSUM") as avpool,
        tc.tile_pool(name="prodp", bufs=1, space="PSUM") as prodpool,
        tc.tile_pool(name="dram", bufs=2, space="DRAM") as dpool,
    ):
        # ---- persistent SBUF tensors ----
        xb_sb = persist.tile([C, n_tok], F32)
        xh_sb = persist.tile([C, n_tok], F16)
        k_sb = persist.tile([C + 1, n_tok], F16)
        q_sb = persist.tile([C + 1, m_tok], F16)
        vt_sb = persist.tile([NT, ntiles * (C + 1)], F16)
        wqh_sb = persist.tile([C, C], F16)
        wql_sb = persist.tile([C, C], F16)
        wkh_sb = persist.tile([C, C + 1], F16)
        wkl_sb = persist.tile([C, C + 1], F16)
        wvhl_sb = persist.tile([C, 2 * C], F16)
        bpc_sb = persist.tile([C, 1], F32)
        pair_sb = persist.tile([C, GROUPS], F32)
        expand_sb = persist.tile([GROUPS, C], F32)
        stats_sb = persist.tile([C, 2 * nch], F32)
        scratch_sb = persist.tile([C, sch], F32)
        scratch2_sb = persist.tile([C, sch], F32)
        eps_sb = persist.tile([GROUPS, 1], F32)
        mrg_sb = persist.tile([GROUPS, 2], F32)
        mrc_sb = persist.tile([C, 2], F32)

        # x first: it gates the whole stats -> normalize -> project chain
        for ch in range(nch):
            sl = slice(ch * sch, (ch + 1) * sch)
            nc.sync.dma_start(out=xb_sb[:, sl], in_=xb_d[:, sl])
        nc.sync.dma_start(out=wqh_sb[:], in_=wqh_d[:, :])
        nc.sync.dma_start(out=wql_sb[:], in_=wql_d[:, :])
        nc.sync.dma_start(out=wkh_sb[:], in_=wkh_d[:, :])
        nc.sync.dma_start(out=wkl_sb[:], in_=wkl_d[:, :])
        nc.sync.dma_start(out=wvhl_sb[:], in_=wvhl_d[:, :])
        nc.sync.dma_start(out=bpc_sb[:], in_=bpc_d[:, :])
        nc.sync.dma_start(out=pair_sb[:], in_=pair_d[:, :])
        nc.sync.dma_start(out=expand_sb[:], in_=expand_d[:, :])
        nc.vector.memset(eps_sb[:], EPS)
        # ones column (col C of each 65-wide v^T block) -> AV rowsum; ones row
        # of q -> bq contribution to scores.
        nc.gpsimd.memset(vt_sb[:], 1.0)
        nc.gpsimd.memset(q_sb[C : C + 1, :], 1.0)

        # ---- per-channel sum / sum-of-squares ----
        for ch in range(nch):
            sl = slice(ch * sch, (ch + 1) * sch)
            nc.vector.tensor_scalar(
                out=scratch2_sb[:], in0=xb_sb[:, sl], scalar1=1.0, scalar2=None,
                op0=OP.mult, op1=OP.add, accum_out=stats_sb[:, ch : ch + 1],
            )
            nc.scalar.activation(
                out=scratch_sb[:], in_=xb_sb[:, sl], func=AF.Square,
                accum_out=stats_sb[:, nch + ch : nch + ch + 1],
            )

        # ---- group statistics: pair-sum across channel pairs + chunks ----
        gp = prodpool.tile([GROUPS, 2 * nch], F32, tag="prod")
        nc.tensor.matmul(gp[:], pair_sb[:], stats_sb[:], start=True, stop=True)
        gsum = mpool.tile([GROUPS, 2], F32, tag="gsum")
        nc.vector.tensor_reduce(
            out=gsum[:], in_=gp[:].rearrange("p (s c) -> p s c", s=2),
            axis=mybir.AxisListType.X, op=OP.add,
        )
        # var = Ex2 - mean^2 ; rstd = 1/sqrt(var + eps) ; keep [mean, rstd]
        msq = mpool.tile([GROUPS, 1], F32, tag="msq")
        nc.vector.tensor_mul(msq[:], gsum[:, 0:1], gsum[:, 0:1])
        nc.vector.tensor_copy(mrg_sb[:, 0:1], gsum[:, 0:1])
        nc.vector.tensor_sub(mrg_sb[:, 1:2], gsum[:, 1:2], msq[:])
        nc.scalar.activation(
            out=mrg_sb[:, 1:2], in_=mrg_sb[:, 1:2], func=AF.Sqrt, bias=eps_sb[:],
        )
        nc.vector.reciprocal(mrg_sb[:, 1:2], mrg_sb[:, 1:2])
        ep = prodpool.tile([C, 2], F32, tag="prod")
        nc.tensor.matmul(ep[:], expand_sb[:], mrg_sb[:], start=True, stop=True)
        nc.vector.tensor_copy(mrc_sb[:], ep[:])

        # ---- normalize / projection helpers (fp16, hi/lo passes) ----
        vt_view = vt_sb[:].rearrange("p (t e) -> p t e", e=C + 1)

        def emit_xh(ch):
            sl = slice(ch * xsz, (ch + 1) * xsz)
            nc.vector.tensor_scalar(
                out=xh_sb[:, sl], in0=xb_sb[:, sl],
                scalar1=mrc_sb[:, 0:1], scalar2=mrc_sb[:, 1:2],
                op0=OP.subtract, op1=OP.mult,
            )

        def proj_q(j):
            sl = slice(j * PCH, (j + 1) * PCH)
            qp = prodpool.tile([C, PCH], F32, tag="prod", name="qp")
            nc.tensor.matmul(qp[:], wqh_sb[:], xh_sb[:, sl], start=True, stop=False)
            nc.tensor.matmul(qp[:], wql_sb[:], xh_sb[:, sl], start=False, stop=True)
            nc.vector.tensor_copy(q_sb[0:C, sl], qp[:])

        def proj_k(j):
            sl = slice(j * PCH, (j + 1) * PCH)
            kp = prodpool.tile([C + 1, PCH], F32, tag="prod", name="kp")
            nc.tensor.matmul(kp[:], wkh_sb[:], xh_sb[:, sl], start=True, stop=False)
            nc.tensor.matmul(kp[:], wkl_sb[:], xh_sb[:, sl], start=False, stop=True)
            nc.vector.tensor_copy(k_sb[:, sl], kp[:])

        def proj_vt(j4):
            # two accumulating [128,64] matmuls (wpv hi then lo) per
            # 128-token tile; hi/lo sum happens in the PSUM accumulator
            t0, tn = j4 * 4, min(4, ntiles - j4 * 4)
            vp = prodpool.tile([NT, tn * C], F32, tag="prod", name="vp")
            for t in range(tn):
                j = t0 + t
                xh_t = xh_sb[:, j * NT : (j + 1) * NT]
                nc.tensor.matmul(
                    vp[:, t * C : (t + 1) * C], xh_t, wvhl_sb[:, 0:C],
                    start=True, stop=False,
                )
                nc.tensor.matmul(
                    vp[:, t * C : (t + 1) * C], xh_t, wvhl_sb[:, C : 2 * C],
                    start=False, stop=True,
                )
            nc.vector.tensor_copy(
                vt_view[:, t0 : t0 + tn, 0:C],
                vp[:].rearrange("p (t e) -> p t e", e=C),
            )

        produced = [0]

        def produce_until(tile_limit):
            need = min(tile_limit // 4, nchunks - 1)
            while produced[0] <= need:
                j = produced[0]
                if j % cpx == 0 and j // cpx > 0:
                    emit_xh(j // cpx)
                proj_k(j)
                proj_vt(j)
                produced[0] += 1

        emit_xh(0)
        for j in range(m_tok // PCH):
            proj_q(j)

        # ---- attention, one 512-query block at a time; k/v production for
        # the first block is interleaved group-by-group, and each block's
        # tail is deferred into the next block so nothing stalls the PE/ACT
        # pipeline ----
        def make_tail(av_sb, msl):
            def tail():
                recip = mpool.tile([1, MB], F32, tag="recip", name="recip")
                nc.vector.reciprocal(recip[:], av_sb[C : C + 1, :])
                # partition-broadcast recip via a DRAM bounce (SBUF-source
                # DMA cannot replicate across partitions)
                rd = dpool.tile([1, MB], F32, tag="rd", name="rd")
                nc.sync.dma_start(out=rd[:], in_=recip[:])
                rb = mpool.tile([C, MB], F32, tag="rb", name="rb")
                nc.sync.dma_start(out=rb[:], in_=rd[:].to_broadcast([C, MB]))
                t1 = mpool.tile([C, MB], F32, tag="t1", name="t1")
                nc.vector.tensor_mul(t1[:], av_sb[0:C, :], rb[:])
                outt = mpool.tile([C, MB], F32, tag="outt", name="outt")
                nc.vector.scalar_tensor_tensor(
                    out=outt[:], in0=t1[:], scalar=bpc_sb[:], in1=xb_sb[:, msl],
                    op0=OP.add, op1=OP.add,
                )
                nc.sync.dma_start(out=out_d[:, msl], in_=outt[:])
            return tail

        deferred = None
        for mb in range(m_tok // MB):
            msl = slice(mb * MB, (mb + 1) * MB)
            av = avpool.tile([C + 1, MB], F32, tag="av")
            pending = None
            for gi, g0 in enumerate(range(0, ntiles, GSZ)):
                gsz = min(GSZ, ntiles - g0)
                if mb == 0:
                    produce_until(min(g0 + 2 * GSZ - 1, ntiles - 1))
                sp = spool.tile([NT, gsz * MB], F32, tag="s")
                for t in range(gsz):
                    j = g0 + t
                    nc.tensor.matmul(
                        sp[:, t * MB : (t + 1) * MB],
                        k_sb[:, j * NT : (j + 1) * NT], q_sb[:, msl],
                        start=True, stop=True,
                    )
                ex = epool.tile([NT, gsz * MB], F16, tag="e")
                nc.scalar.activation(out=ex[:], in_=sp[:], func=AF.Exp)
                if pending is not None:
                    pg0, psz, pex = pending
                    for t in range(psz):
                        j = pg0 + t
                        nc.tensor.matmul(
                            av[:], vt_view[:, j, :], pex[:, t * MB : (t + 1) * MB],
                            start=(j == 0), stop=(j == ntiles - 1),
                        )
                pending = (g0, gsz, ex)
                if gi == 3 and deferred is not None:
                    deferred()
                    deferred = None
            pg0, psz, pex = pending
            for t in range(psz):
                j = pg0 + t
                nc.tensor.matmul(
                    av[:], vt_view[:, j, :], pex[:, t * MB : (t + 1) * MB],
                    start=(j == 0), stop=(j == ntiles - 1),
                )
            # evacuate immediately so the (bufs=1) accumulator bank frees up
            av_sb = mpool.tile([C + 1, MB], F32, tag="avsb", name="av_sb")
            nc.vector.tensor_copy(av_sb[:], av[:])
            if deferred is not None:  # few-group case: gi==3 never fired
                deferred()
            deferred = make_tail(av_sb, msl)
        deferred()


def build_program(n_tok=N_FULL, m_tok=M_FULL):
    nc = bacc.Bacc("TRN2", target_bir_lowering=False, debug=False)
    xb_d = nc.dram_tensor("xb", [C, n_tok], F32, kind="ExternalInput")
    wqh_d = nc.dram_tensor("wqh", [C, C], F16, kind="ExternalInput")
    wql_d = nc.dram_tensor("wql", [C, C], F16, kind="ExternalInput")
    wkh_d = nc.dram_tensor("wkh", [C, C + 1], F16, kind="ExternalInput")
    wkl_d = nc.dram_tensor("wkl", [C, C + 1], F16, kind="ExternalInput")
    wvhl_d = nc.dram_tensor("wvhl", [C, 2 * C], F16, kind="ExternalInput")
    bpc_d = nc.dram_tensor("bpc", [C, 1], F32, kind="ExternalInput")
    pair_d = nc.dram_tensor("pair", [C, GROUPS], F32, kind="ExternalInput")
    expand_d = nc.dram_tensor("expand", [GROUPS, C], F32, kind="ExternalInput")
    out_d = nc.dram_tensor("out", [C, m_tok], F32, kind="ExternalOutput")
    with tile.TileContext(nc) as tc:
        emit(tc, nc, n_tok, m_tok,
             xb_d.ap(), wqh_d.ap(), wql_d.ap(), wkh_d.ap(), wkl_d.ap(),
             wvhl_d.ap(), bpc_d.ap(), pair_d.ap(), expand_d.ap(), out_d.ap())
    nc.compile()
    return nc


def prep_weights(gamma, beta, wq, bq, wk, bk, wv, bv, wp, bp, n_tok=N_FULL):
    """Host-side algebraic folds. Returns the shared per-core input dict."""
    f32 = np.float32
    gamma, beta = gamma.astype(f32), beta.astype(f32)
    scale = f32(1.0) / np.sqrt(f32(C)).astype(f32)
    wq_eff = (wq * gamma[None, :]) * scale
    bq_eff = (wq @ beta + bq) * scale
    wk_eff = wk * gamma[None, :]
    wv_eff = wv * gamma[None, :]
    bv_eff = wv @ beta + bv
    bp_eff = (bp + wp @ bv_eff).astype(f32)
    wpv_eff = (wp @ wv_eff).astype(f32)  # wp folded into v

    wkT = np.zeros((C, C + 1), f32)
    wkT[:, 0:C] = wk_eff.T
    wkT[:, C] = wk_eff.T @ bq_eff
    pair = np.zeros((C, GROUPS), f32)
    pair[np.arange(C), np.arange(C) // 2] = f32(1.0) / f32(2 * n_tok)
    expand = np.zeros((GROUPS, C), f32)
    expand[np.arange(C) // 2, np.arange(C)] = 1.0

    def split16(a):
        hi = a.astype(np.float16)
        lo = (a - hi.astype(f32)).astype(np.float16)
        return hi, lo

    wqh, wql = split16(np.ascontiguousarray(wq_eff.T, f32))
    wkh, wkl = split16(np.ascontiguousarray(wkT, f32))
    wvh, wvl = split16(np.ascontiguousarray(wpv_eff.T, f32))
    return {
        "wqh": wqh, "wql": wql,
        "wkh": wkh, "wkl": wkl,
        "wvhl": np.ascontiguousarray(np.concatenate([wvh, wvl], axis=1)),
        "bpc": bp_eff.reshape(C, 1),
        "pair": pair,
        "expand": expand,
    }


_PROGRAM_CACHE = {}


def _get_program(n_tok, m_tok):
    key = (n_tok, m_tok)
    if key not in _PROGRAM_CACHE:
        _PROGRAM_CACHE[key] = build_program(n_tok, m_tok)
    return _PROGRAM_CACHE[key]


def make_in_maps(x, shared):
    """Per-core input maps: batch b = core//4, query chunk qc = core%4."""
    in_maps = []
    for core in range(N_CORES):
        b, qc = core // Q_CHUNKS, core % Q_CHUNKS
        xb = np.ascontiguousarray(x[b].reshape(C, N_FULL), np.float32)
        xb = np.ascontiguousarray(np.roll(xb, -qc * M_FULL, axis=1))
        in_maps.append({"xb": xb, **shared})
    return in_maps


def kernel(x, gamma, beta, wq, bq, wk, bk, wv, bv, wp, bp, **run_kwargs):
    from concourse.bass_utils import run_bass_kernel_spmd

    x = np.asarray(x, np.float32)
    shared = prep_weights(
        np.asarray(gamma), np.asarray(beta), np.asarray(wq), np.asarray(bq),
        np.asarray(wk), np.asarray(bk), np.asarray(wv), np.asarray(bv),
        np.asarray(wp), np.asarray(bp),
    )
    nc = _get_program(N_FULL, M_FULL)
    in_maps = make_in_maps(x, shared)
    res = run_bass_kernel_spmd(nc, in_maps, core_ids=list(range(N_CORES)), **run_kwargs)
    y = np.empty((B_FULL, C, N_FULL), np.float32)
    for core in range(N_CORES):
        b, qc = core // Q_CHUNKS, core % Q_CHUNKS
        y[b, :, qc * M_FULL : (qc + 1) * M_FULL] = res.results[core]["out"]
    out = y.reshape(B_FULL, C, H_FULL, W_FULL, D_FULL)
    if run_kwargs:
        return out, res
    return out



# revision 4
# speedup vs baseline: 1.0052x; 1.0052x over previous
"""Trainium2 Bass kernel for MemoryEfficientAttnBlock3D.

Computes: y = x + conv1x1(attn(conv1x1_{q,k,v}(groupnorm(x))), wp, bp)
for x of shape (2, 64, 32, 32, 8)  (B=2, C=64, N=8192 tokens per batch).

Sharding: 8 cores = 2 batches x 4 query-chunks of 2048 tokens.  Each core
receives its batch's full token volume ROTATED so that its query chunk is
always tokens [0:2048] -- groupnorm statistics and softmax/AV reductions are
permutation-invariant over kv tokens, so all cores run an identical program.

Algebraic folds done on the host:
  - gamma folds into Wq/Wk/Wv columns; the attention scale 1/sqrt(C) into Wq.
  - k's additive constant (Wk@beta + bk) shifts every score of a softmax row
    equally -> dropped exactly (softmax shift invariance).
  - bq enters scores via an extra contraction row: q carries a constant ones
    row, k an extra output row ((Wk_eff^T @ bq_eff) @ xn).
  - the OUTPUT projection wp folds into the v weights: sum_c wp[o,c] v[c,n]
    = (wp@Wv_eff) @ xn[:,n], so the AV matmul accumulates wp@AV directly and
    the standalone P matmul disappears.
  - v's additive constant and bp are applied as a per-partition scalar in
    the final fused (t + bp) + x DVE op.
  - softmax denominators: v^T carries a ones column, so the AV matmul
    accumulates [wp@AV | rowsum]; the division is applied at the very end
    (column scaling commutes with everything linear).

Precision: all attention / projection matmuls run in fp16 (fp32 matmuls are
4x slower on the PE: LOW_HIGH weight split x half-rate fp32 streaming, and
fp32 streams do not lift the HAM clock gate).  Projection weights are sent
as fp16 hi/lo pairs and applied in two accumulating passes, which removes
the weight-rounding error; groupnorm, statistics, softmax scores (PSUM),
rowsums and the final combine stay fp32.  Measured end-to-end absmax vs the
fp32 reference: 4.4e-4 on outputs with |out|max 5.3 (8.4e-5 of scale; the
all-fp32 floor is 1.5e-5, a bf16-native implementation would be 3e-3).
"""

import numpy as np

import concourse.bass as bass
import concourse.tile as tile
from concourse import bacc, mybir

F32 = mybir.dt.float32
F16 = mybir.dt.float16
AF = mybir.ActivationFunctionType
OP = mybir.AluOpType

C = 64
GROUPS = 32
EPS = 1e-6

B_FULL = 2
H_FULL, W_FULL, D_FULL = 32, 32, 8
N_FULL = H_FULL * W_FULL * D_FULL  # 8192 kv tokens per batch
N_CORES = 8
Q_CHUNKS = 4
M_FULL = N_FULL // Q_CHUNKS  # 2048 q tokens per core

MB = 512        # q-token block (one PSUM bank of fp32)
NT = 128        # kv-token tile (matmul M / partition dim)
GSZ = 3         # n-tiles per exp group ([128, 1536] PSUM tile = 3 banks)
STAT_CHUNK = 1024
PCH = 512       # projection chunk (tokens)


def emit(tc, nc, n_tok, m_tok, xb_d, wqh_d, wql_d, wkh_d, wkl_d, wvhl_d,
         bpc_d, pair_d, expand_d, out_d):
    ntiles = n_tok // NT
    nch = max(1, n_tok // STAT_CHUNK)
    sch = n_tok // nch
    nchunks = n_tok // PCH
    xch = max(1, n_tok // 2048)   # xh (normalize) macro-chunks
    xsz = n_tok // xch
    cpx = xsz // PCH              # projection chunks per xh chunk

    with (
        tc.tile_pool(name="persist", bufs=1) as persist,
        tc.tile_pool(name="expS", bufs=4) as epool,
        tc.tile_pool(name="mtail", bufs=3) as mpool,
        tc.tile_pool(name="spsum", bufs=2, space="PSUM") as spool,
        tc.tile_pool(name="avpsum", bufs=1, space="PSUM") as avpool,
        tc.tile_pool(name="prodp", bufs=1, space="PSUM") as prodpool,
        tc.tile_pool(name="dram", bufs=2, space="DRAM") as dpool,
    ):
        # ---- persistent SBUF tensors ----
        xb_sb = persist.tile([C, n_tok], F32)
        xh_sb = persist.tile([C, n_tok], F16)
        k_sb = persist.tile([C + 1, n_tok], F16)
        q_sb = persist.tile([C + 1, m_tok], F16)
        vt_sb = persist.tile([NT, ntiles * (C + 1)], F16)
        wqh_sb = persist.tile([C, C], F16)
        wql_sb = persist.tile([C, C], F16)
        wkh_sb = persist.tile([C, C + 1], F16)
        wkl_sb = persist.tile([C, C + 1], F16)
        wvhl_sb = persist.tile([C, 2 * C], F16)
        bpc_sb = persist.tile([C, 1], F32)
        pair_sb = persist.tile([C, GROUPS], F32)
        expand_sb = persist.tile([GROUPS, C], F32)
        stats_sb = persist.tile([C, 2 * nch], F32)
        scratch_sb = persist.tile([C, sch], F32)
        scratch2_sb = persist.tile([C, sch], F32)
        eps_sb = persist.tile([GROUPS, 1], F32)
        mrg_sb = persist.tile([GROUPS, 2], F32)
        mrc_sb = persist.tile([C, 2], F32)

        # ---- PE warm-up: the attention matmuls light up only ~51% of the
        # array (K=65 or M=65 of 128), which never trips HAM's activity
        # monitor -- the baseline ran its first ~119us at the cold 1.2 GHz
        # clock.  A short burst of full-array fp16 matmuls on memset data
        # flips the clock gate to 8/8 (~3.4us sustained busy) while the
        # input DMA is still in flight, so everything after runs at 2.4 GHz.
        warm_sb = persist.tile([128, 512], F16)
        nc.vector.memset(warm_sb[:], 0.25)
        for wi in range(12):
            warm_ps = prodpool.tile([128, 512], F32, tag="prod", name="warm_ps")
            nc.tensor.matmul(
                warm_ps[:], warm_sb[:, 0:128], warm_sb[:], start=True, stop=True,
            )

        # ---- ACT table preloads: pull the sqrt-set load into the DMA
        # shadow at t=0 (the stats Squares then run under it), and load the
        # exp set right after the real Sqrt so it overlaps normalize/proj
        # instead of stalling the first real exp.
        nc.vector.memset(eps_sb[:], EPS)
        actscr = persist.tile([GROUPS, 1], F32)
        nc.scalar.activation(out=actscr[:], in_=eps_sb[:], func=AF.Sqrt)

        # x first: it gates the whole stats -> normalize -> project chain
        for ch in range(nch):
            sl = slice(ch * sch, (ch + 1) * sch)
            nc.sync.dma_start(out=xb_sb[:, sl], in_=xb_d[:, sl])
        nc.sync.dma_start(out=wqh_sb[:], in_=wqh_d[:, :])
        nc.sync.dma_start(out=wql_sb[:], in_=wql_d[:, :])
        nc.sync.dma_start(out=wkh_sb[:], in_=wkh_d[:, :])
        nc.sync.dma_start(out=wkl_sb[:], in_=wkl_d[:, :])
        nc.sync.dma_start(out=wvhl_sb[:], in_=wvhl_d[:, :])
        nc.sync.dma_start(out=bpc_sb[:], in_=bpc_d[:, :])
        nc.sync.dma_start(out=pair_sb[:], in_=pair_d[:, :])
        nc.sync.dma_start(out=expand_sb[:], in_=expand_d[:, :])
        # ones column (col C of each 65-wide v^T block) -> AV rowsum; ones row
        # of q -> bq contribution to scores.
        nc.gpsimd.memset(vt_sb[:], 1.0)
        nc.gpsimd.memset(q_sb[C : C + 1, :], 1.0)

        # ---- per-channel sum / sum-of-squares ----
        for ch in range(nch):
            sl = slice(ch * sch, (ch + 1) * sch)
            nc.vector.tensor_scalar(
                out=scratch2_sb[:], in0=xb_sb[:, sl], scalar1=1.0, scalar2=None,
                op0=OP.mult, op1=OP.add, accum_out=stats_sb[:, ch : ch + 1],
            )
            nc.scalar.activation(
                out=scratch_sb[:], in_=xb_sb[:, sl], func=AF.Square,
                accum_out=stats_sb[:, nch + ch : nch + ch + 1],
            )

        # ---- group statistics: pair-sum across channel pairs + chunks ----
        gp = prodpool.tile([GROUPS, 2 * nch], F32, tag="prod")
        nc.tensor.matmul(gp[:], pair_sb[:], stats_sb[:], start=True, stop=True)
        gsum = mpool.tile([GROUPS, 2], F32, tag="gsum")
        nc.vector.tensor_reduce(
            out=gsum[:], in_=gp[:].rearrange("p (s c) -> p s c", s=2),
            axis=mybir.AxisListType.X, op=OP.add,
        )
        # var = Ex2 - mean^2 ; rstd = 1/sqrt(var + eps) ; keep [mean, rstd]
        msq = mpool.tile([GROUPS, 1], F32, tag="msq")
        nc.vector.tensor_mul(msq[:], gsum[:, 0:1], gsum[:, 0:1])
        nc.vector.tensor_copy(mrg_sb[:, 0:1], gsum[:, 0:1])
        nc.vector.tensor_sub(mrg_sb[:, 1:2], gsum[:, 1:2], msq[:])
        nc.scalar.activation(
            out=mrg_sb[:, 1:2], in_=mrg_sb[:, 1:2], func=AF.Sqrt, bias=eps_sb[:],
        )
        nc.vector.reciprocal(mrg_sb[:, 1:2], mrg_sb[:, 1:2])
        # preload the exp table set now; overlaps normalize/projection
        nc.scalar.activation(out=actscr[:], in_=eps_sb[:], func=AF.Exp)
        ep = prodpool.tile([C, 2], F32, tag="prod")
        nc.tensor.matmul(ep[:], expand_sb[:], mrg_sb[:], start=True, stop=True)
        nc.vector.tensor_copy(mrc_sb[:], ep[:])

        # ---- normalize / projection helpers (fp16, hi/lo passes) ----
        vt_view = vt_sb[:].rearrange("p (t e) -> p t e", e=C + 1)

        def emit_xh(ch):
            sl = slice(ch * xsz, (ch + 1) * xsz)
            nc.vector.tensor_scalar(
                out=xh_sb[:, sl], in0=xb_sb[:, sl],
                scalar1=mrc_sb[:, 0:1], scalar2=mrc_sb[:, 1:2],
                op0=OP.subtract, op1=OP.mult,
            )

        def proj_q(j):
            sl = slice(j * PCH, (j + 1) * PCH)
            qp = prodpool.tile([C, PCH], F32, tag="prod", name="qp")
            nc.tensor.matmul(qp[:], wqh_sb[:], xh_sb[:, sl], start=True, stop=False)
            nc.tensor.matmul(qp[:], wql_sb[:], xh_sb[:, sl], start=False, stop=True)
            nc.vector.tensor_copy(q_sb[0:C, sl], qp[:])

        def proj_k(j):
            sl = slice(j * PCH, (j + 1) * PCH)
            kp = prodpool.tile([C + 1, PCH], F32, tag="prod", name="kp")
            nc.tensor.matmul(kp[:], wkh_sb[:], xh_sb[:, sl], start=True, stop=False)
            nc.tensor.matmul(kp[:], wkl_sb[:], xh_sb[:, sl], start=False, stop=True)
            nc.vector.tensor_copy(k_sb[:, sl], kp[:])

        def proj_vt(j4):
            # two accumulating [128,64] matmuls (wpv hi then lo) per
            # 128-token tile; hi/lo sum happens in the PSUM accumulator
            t0, tn = j4 * 4, min(4, ntiles - j4 * 4)
            vp = prodpool.tile([NT, tn * C], F32, tag="prod", name="vp")
            for t in range(tn):
                j = t0 + t
                xh_t = xh_sb[:, j * NT : (j + 1) * NT]
                nc.tensor.matmul(
                    vp[:, t * C : (t + 1) * C], xh_t, wvhl_sb[:, 0:C],
                    start=True, stop=False,
                )
                nc.tensor.matmul(
                    vp[:, t * C : (t + 1) * C], xh_t, wvhl_sb[:, C : 2 * C],
                    start=False, stop=True,
                )
            nc.vector.tensor_copy(
                vt_view[:, t0 : t0 + tn, 0:C],
                vp[:].rearrange("p (t e) -> p t e", e=C),
            )

        produced = [0]

        def produce_until(tile_limit):
            need = min(tile_limit // 4, nchunks - 1)
            while produced[0] <= need:
                j = produced[0]
                if j % cpx == 0 and j // cpx > 0:
                    emit_xh(j // cpx)
                proj_k(j)
                proj_vt(j)
                produced[0] += 1

        emit_xh(0)
        for j in range(m_tok // PCH):
            proj_q(j)

        # ---- attention, one 512-query block at a time; k/v production for
        # the first block is interleaved group-by-group, and each block's
        # tail is deferred into the next block so nothing stalls the PE/ACT
        # pipeline ----
        def make_tail(av_sb, msl):
            def tail():
                recip = mpool.tile([1, MB], F32, tag="recip", name="recip")
                nc.vector.reciprocal(recip[:], av_sb[C : C + 1, :])
                # partition-broadcast recip via a DRAM bounce (SBUF-source
                # DMA cannot replicate across partitions)
                rd = dpool.tile([1, MB], F32, tag="rd", name="rd")
                nc.sync.dma_start(out=rd[:], in_=recip[:])
                rb = mpool.tile([C, MB], F32, tag="rb", name="rb")
                nc.sync.dma_start(out=rb[:], in_=rd[:].to_broadcast([C, MB]))
                t1 = mpool.tile([C, MB], F32, tag="t1", name="t1")
                nc.vector.tensor_mul(t1[:], av_sb[0:C, :], rb[:])
                outt = mpool.tile([C, MB], F32, tag="outt", name="outt")
                nc.vector.scalar_tensor_tensor(
                    out=outt[:], in0=t1[:], scalar=bpc_sb[:], in1=xb_sb[:, msl],
                    op0=OP.add, op1=OP.add,
                )
                nc.sync.dma_start(out=out_d[:, msl], in_=outt[:])
            return tail

        deferred = None
        for mb in range(m_tok // MB):
            msl = slice(mb * MB, (mb + 1) * MB)
            av = avpool.tile([C + 1, MB], F32, tag="av")
            pending = None
            for gi, g0 in enumerate(range(0, ntiles, GSZ)):
                gsz = min(GSZ, ntiles - g0)
                if mb == 0:
                    produce_until(min(g0 + 2 * GSZ - 1, ntiles - 1))
                sp = spool.tile([NT, gsz * MB], F32, tag="s")
                for t in range(gsz):
                    j = g0 + t
                    nc.tensor.matmul(
                        sp[:, t * MB : (t + 1) * MB],
                        k_sb[:, j * NT : (j + 1) * NT], q_sb[:, msl],
                        start=True, stop=True,
                    )
                ex = epool.tile([NT, gsz * MB], F16, tag="e")
                nc.scalar.activation(out=ex[:], in_=sp[:], func=AF.Exp)
                if pending is not None:
                    pg0, psz, pex = pending
                    for t in range(psz):
                        j = pg0 + t
                        nc.tensor.matmul(
                            av[:], vt_view[:, j, :], pex[:, t * MB : (t + 1) * MB],
                            start=(j == 0), stop=(j == ntiles - 1),
                        )
                pending = (g0, gsz, ex)
                if gi == 3 and deferred is not None:
                    deferred()
                    deferred = None
            pg0, psz, pex = pending
            for t in range(psz):
                j = pg0 + t
                nc.tensor.matmul(
                    av[:], vt_view[:, j, :], pex[:, t * MB : (t + 1) * MB],
                    start=(j == 0), stop=(j == ntiles - 1),
                )
            # evacuate immediately so the (bufs=1) accumulator bank frees up
            av_sb = mpool.tile([C + 1, MB], F32, tag="avsb", name="av_sb")
            nc.vector.tensor_copy(av_sb[:], av[:])
            if deferred is not None:  # few-group case: gi==3 never fired
                deferred()
            deferred = make_tail(av_sb, msl)
        deferred()


def build_program(n_tok=N_FULL, m_tok=M_FULL):
    nc = bacc.Bacc("TRN2", target_bir_lowering=False, debug=False)
    xb_d = nc.dram_tensor("xb", [C, n_tok], F32, kind="ExternalInput")
    wqh_d = nc.dram_tensor("wqh", [C, C], F16, kind="ExternalInput")
    wql_d = nc.dram_tensor("wql", [C, C], F16, kind="ExternalInput")
    wkh_d = nc.dram_tensor("wkh", [C, C + 1], F16, kind="ExternalInput")
    wkl_d = nc.dram_tensor("wkl", [C, C + 1], F16, kind="ExternalInput")
    wvhl_d = nc.dram_tensor("wvhl", [C, 2 * C], F16, kind="ExternalInput")
    bpc_d = nc.dram_tensor("bpc", [C, 1], F32, kind="ExternalInput")
    pair_d = nc.dram_tensor("pair", [C, GROUPS], F32, kind="ExternalInput")
    expand_d = nc.dram_tensor("expand", [GROUPS, C], F32, kind="ExternalInput")
    out_d = nc.dram_tensor("out", [C, m_tok], F32, kind="ExternalOutput")
    with tile.TileContext(nc) as tc:
        emit(tc, nc, n_tok, m_tok,
             xb_d.ap(), wqh_d.ap(), wql_d.ap(), wkh_d.ap(), wkl_d.ap(),
             wvhl_d.ap(), bpc_d.ap(), pair_d.ap(), expand_d.ap(), out_d.ap())
    nc.compile()
    return nc


def prep_weights(gamma, beta, wq, bq, wk, bk, wv, bv, wp, bp, n_tok=N_FULL):
    """Host-side algebraic folds. Returns the shared per-core input dict."""
    f32 = np.float32
    gamma, beta = gamma.astype(f32), beta.astype(f32)
    scale = f32(1.0) / np.sqrt(f32(C)).astype(f32)
    wq_eff = (wq * gamma[None, :]) * scale
    bq_eff = (wq @ beta + bq) * scale
    wk_eff = wk * gamma[None, :]
    wv_eff = wv * gamma[None, :]
    bv_eff = wv @ beta + bv
    bp_eff = (bp + wp @ bv_eff).astype(f32)
    wpv_eff = (wp @ wv_eff).astype(f32)  # wp folded into v

    wkT = np.zeros((C, C + 1), f32)
    wkT[:, 0:C] = wk_eff.T
    wkT[:, C] = wk_eff.T @ bq_eff
    pair = np.zeros((C, GROUPS), f32)
    pair[np.arange(C), np.arange(C) // 2] = f32(1.0) / f32(2 * n_tok)
    expand = np.zeros((GROUPS, C), f32)
    expand[np.arange(C) // 2, np.arange(C)] = 1.0

    def split16(a):
        hi = a.astype(np.float16)
        lo = (a - hi.astype(f32)).astype(np.float16)
        return hi, lo

    wqh, wql = split16(np.ascontiguousarray(wq_eff.T, f32))
    wkh, wkl = split16(np.ascontiguousarray(wkT, f32))
    wvh, wvl = split16(np.ascontiguousarray(wpv_eff.T, f32))
    return {
        "wqh": wqh, "wql": wql,
        "wkh": wkh, "wkl": wkl,
        "wvhl": np.ascontiguousarray(np.concatenate([wvh, wvl], axis=1)),
        "bpc": bp_eff.reshape(C, 1),
        "pair": pair,
        "expand": expand,
    }


_PROGRAM_CACHE = {}


def _get_program(n_tok, m_tok):
    key = (n_tok, m_tok)
    if key not in _PROGRAM_CACHE:
        _PROGRAM_CACHE[key] = build_program(n_tok, m_tok)
    return _PROGRAM_CACHE[key]


def make_in_maps(x, shared):
    """Per-core input maps: batch b = core//4, query chunk qc = core%4."""
    in_maps = []
    for core in range(N_CORES):
        b, qc = core // Q_CHUNKS, core % Q_CHUNKS
        xb = np.ascontiguousarray(x[b].reshape(C, N_FULL), np.float32)
        xb = np.ascontiguousarray(np.roll(xb, -qc * M_FULL, axis=1))
        in_maps.append({"xb": xb, **shared})
    return in_maps


def kernel(x, gamma, beta, wq, bq, wk, bk, wv, bv, wp, bp, **run_kwargs):
    from concourse.bass_utils import run_bass_kernel_spmd

    x = np.asarray(x, np.float32)
    shared = prep_weights(
        np.asarray(gamma), np.asarray(beta), np.asarray(wq), np.asarray(bq),
        np.asarray(wk), np.asarray(bk), np.asarray(wv), np.asarray(bv),
        np.asarray(wp), np.asarray(bp),
    )
    nc = _get_program(N_FULL, M_FULL)
    in_maps = make_in_maps(x, shared)
    res = run_bass_kernel_spmd(nc, in_maps, core_ids=list(range(N_CORES)), **run_kwargs)
    y = np.empty((B_FULL, C, N_FULL), np.float32)
    for core in range(N_CORES):
        b, qc = core // Q_CHUNKS, core % Q_CHUNKS
        y[b, :, qc * M_FULL : (qc + 1) * M_FULL] = res.results[core]["out"]
    out = y.reshape(B_FULL, C, H_FULL, W_FULL, D_FULL)
    if run_kwargs:
        return out, res
    return out



# revision 12
# speedup vs baseline: 1.0970x; 1.0912x over previous
"""Trainium2 Bass kernel for MemoryEfficientAttnBlock3D.

Computes: y = x + conv1x1(attn(conv1x1_{q,k,v}(groupnorm(x))), wp, bp)
for x of shape (2, 64, 32, 32, 8)  (B=2, C=64, N=8192 tokens per batch).

Sharding: 8 cores = 2 batches x 4 query-chunks of 2048 tokens.  Each core
receives its batch's full token volume ROTATED so that its query chunk is
always tokens [0:2048] -- groupnorm statistics and softmax/AV reductions are
permutation-invariant over kv tokens, so all cores run an identical program.

Algebraic folds done on the host:
  - gamma folds into Wq/Wk/Wv columns; the attention scale 1/sqrt(C) into Wq.
  - k's additive constant (Wk@beta + bk) shifts every score of a softmax row
    equally -> dropped exactly (softmax shift invariance).
  - bq enters via a shift of the normalized activations: xh' = xn + c with
    Wq_eff@c = bq_eff.  q picks up bq exactly; k picks up a per-q-row score
    shift (softmax-invariant, cancels between numerator and denominator);
    v picks up Wpv@c which is subtracted from the output bias.  (For this
    problem bq_eff == 0 so c == 0.)
  - the output projection wp folds into the v weights (wp@Wv_eff), so the AV
    matmul accumulates wp@AV directly.
  - v's additive constant and bp are applied as a per-partition scalar in
    the final fused (t + bp) + x DVE op.

Tensor-engine packing (the attention matmuls only use half the 128x128 PE
array naively: K=64 contraction for scores, M=64 output for AV):
  - scores are ROW-TILED 2x: kv tiles are packed in pairs into k2
    [128, n/2] (tile 2p on partitions 0:64, tile 2p+1 on 64:128) and q is
    duplicated into both partition halves; two concurrent K=64 matmuls
    (tile_position (0,0) and (64,0)) produce two score tiles per 256-column
    stream.
  - AV keeps the proven 65-wide form (vt carries a ones column whose
    output row accumulates the softmax denominators); col-tiled partition-
    offset PSUM outputs are not lowered correctly by this stack (measured:
    device fault + CoreSim zero-region corruption), so M=65 stays.
  - projections stack the fp16 hi/lo weight split along the contraction
    (xh duplicated into both halves), halving matmul count vs two
    accumulating passes.

The PE runs at the cold 1.2 GHz p-state for the first ~117us of every
execution (power ramp; activity does NOT lift it early -- measured), so PE
work is minimized to keep the kernel ACT-bound: the ~16.8M softmax exps per
core on the Scalar engine (1 elem/lane/cycle @ 1.2 GHz, dtype-independent)
are the hard floor (~110us).
"""

import numpy as np

import concourse.bass as bass
import concourse.tile as tile
from concourse import bacc, bass_isa, mybir

F32 = mybir.dt.float32
F16 = mybir.dt.float16
AF = mybir.ActivationFunctionType
OP = mybir.AluOpType

C = 64
GROUPS = 32
EPS = 1e-6

B_FULL = 2
H_FULL, W_FULL, D_FULL = 32, 32, 8
N_FULL = H_FULL * W_FULL * D_FULL  # 8192 kv tokens per batch
N_CORES = 8
Q_CHUNKS = 4
M_FULL = N_FULL // Q_CHUNKS  # 2048 q tokens per core

MB = 256        # q-token block
NT = 128        # kv-token tile (matmul M / partition dim)
GSZ = 6         # kv tiles per exp group ([128, 1536] PSUM tile = 3 banks)
STAT_CHUNK = 1024
PCH = 512       # projection chunk (tokens)


def emit(tc, nc, n_tok, m_tok, xb_d, wq4_d, wk4_d, wv2_d, bpc_d, c2_d,
         pair_d, expand2_d, out_d):
    ntiles = n_tok // NT          # 64
    npairs = ntiles // 2          # 32
    ngroups = (ntiles + GSZ - 1) // GSZ  # 11
    nch = max(1, n_tok // STAT_CHUNK)
    sch = n_tok // nch
    nkch = n_tok // PCH           # 16 k-projection chunks
    nvb = ntiles // 8             # 8 vt-projection batches
    xch = max(1, n_tok // 2048)   # 4 normalize macro-chunks
    xsz = n_tok // xch
    nblk = m_tok // MB            # 8

    def gsize(g):
        return min(GSZ, ntiles - g * GSZ)

    with (
        tc.tile_pool(name="persist", bufs=1) as persist,
        tc.tile_pool(name="expS", bufs=4) as epool,
        tc.tile_pool(name="mtail", bufs=3) as mpool,
        tc.tile_pool(name="spsum", bufs=2, space="PSUM") as spool,
        tc.tile_pool(name="avpsum", bufs=1, space="PSUM") as avpool,
        tc.tile_pool(name="prodp", bufs=1, space="PSUM") as prodpool,
        tc.tile_pool(name="dram", bufs=2, space="DRAM") as dpool,
    ):
        # ---- persistent SBUF tensors ----
        xb2_sb = persist.tile([2 * C, n_tok], F32)
        xh2_sb = persist.tile([2 * C, n_tok], F16)   # normalized, dup halves
        k2_sb = persist.tile([2 * C, n_tok // 2], F16)  # kv tile pairs
        q2_sb = persist.tile([2 * C, m_tok], F16)    # dup halves
        vt_sb = persist.tile([NT, ntiles * (C + 1)], F16)
        wq4_sb = persist.tile([2 * C, 2 * C], F16)
        wk4_sb = persist.tile([2 * C, 2 * C], F16)
        wv2_sb = persist.tile([2 * C, C], F16)
        bpc_sb = persist.tile([C, 1], F32)
        c2_sb = persist.tile([2 * C, 1], F32)
        pair_sb = persist.tile([C, GROUPS], F32)
        expand2_sb = persist.tile([GROUPS, 2 * C], F32)
        stats_sb = persist.tile([C, 2 * nch], F32)
        scratch_sb = persist.tile([C, sch], F32)
        scratch2_sb = persist.tile([C, sch], F32)
        eps_sb = persist.tile([GROUPS, 1], F32)
        mrg_sb = persist.tile([GROUPS, 3], F32)      # [mean, rstd, sd]
        mrc2_sb = persist.tile([2 * C, 3], F32)
        tmpc_sb = persist.tile([2 * C, 1], F32)
        warm_sb = persist.tile([NT, 512], F16)
        actscr = persist.tile([GROUPS, 1], F32)

        # ---- PE warm-up burst + ACT table preloads at t=0 ----
        nc.vector.memset(warm_sb[:], 0.25)
        nc.vector.memset(eps_sb[:], EPS)
        for _ in range(10):
            warm_ps = prodpool.tile([NT, 512], F32, tag="prod", name="warm_ps")
            nc.tensor.matmul(
                warm_ps[:], warm_sb[:, 0:NT], warm_sb[:], start=True, stop=True,
            )
        # sqrt table set loads in the DMA shadow; Square runs under it
        nc.scalar.activation(out=actscr[:], in_=eps_sb[:], func=AF.Sqrt)

        # x first: it gates the whole stats -> normalize -> project chain
        # (duplicated into both partition halves: engines cannot address
        # different base partitions in one op, so the bottom-half normalize
        # needs its own copy of x)
        for ch in range(nch):
            sl = slice(ch * sch, (ch + 1) * sch)
            nc.sync.dma_start(out=xb2_sb[0:C, sl], in_=xb_d[:, sl])
        for ch in range(xch):
            sl = slice(ch * xsz, (ch + 1) * xsz)
            nc.sync.dma_start(out=xb2_sb[C : 2 * C, sl], in_=xb_d[:, sl])
        nc.sync.dma_start(out=wq4_sb[:], in_=wq4_d[:, :])
        nc.sync.dma_start(out=wk4_sb[:], in_=wk4_d[:, :])
        nc.sync.dma_start(out=wv2_sb[:], in_=wv2_d[:, :])
        nc.sync.dma_start(out=bpc_sb[:], in_=bpc_d[:, :])
        nc.sync.dma_start(out=c2_sb[:], in_=c2_d[:, :])
        nc.sync.dma_start(out=pair_sb[:], in_=pair_d[:, :])
        nc.sync.dma_start(out=expand2_sb[:], in_=expand2_d[:, :])
        # ones column (col C of each 65-wide v^T block) -> AV rowsum
        nc.gpsimd.memset(vt_sb[:], 1.0)

        # ---- per-channel sum / sum-of-squares ----
        for ch in range(nch):
            sl = slice(ch * sch, (ch + 1) * sch)
            nc.vector.tensor_scalar(
                out=scratch2_sb[:], in0=xb2_sb[0:C, sl], scalar1=1.0, scalar2=None,
                op0=OP.mult, op1=OP.add, accum_out=stats_sb[:, ch : ch + 1],
            )
            nc.scalar.activation(
                out=scratch_sb[:], in_=xb2_sb[0:C, sl], func=AF.Square,
                accum_out=stats_sb[:, nch + ch : nch + ch + 1],
            )

        # ---- group statistics ----
        gp = prodpool.tile([GROUPS, 2 * nch], F32, tag="prod")
        nc.tensor.matmul(gp[:], pair_sb[:], stats_sb[:], start=True, stop=True)
        gsum = mpool.tile([GROUPS, 2], F32, tag="gsum")
        nc.vector.tensor_reduce(
            out=gsum[:], in_=gp[:].rearrange("p (s c) -> p s c", s=2),
            axis=mybir.AxisListType.X, op=OP.add,
        )
        msq = mpool.tile([GROUPS, 1], F32, tag="msq")
        nc.vector.tensor_mul(msq[:], gsum[:, 0:1], gsum[:, 0:1])
        nc.vector.tensor_copy(mrg_sb[:, 0:1], gsum[:, 0:1])
        nc.vector.tensor_sub(mrg_sb[:, 2:3], gsum[:, 1:2], msq[:])
        nc.scalar.activation(
            out=mrg_sb[:, 2:3], in_=mrg_sb[:, 2:3], func=AF.Sqrt, bias=eps_sb[:],
        )
        nc.vector.reciprocal(mrg_sb[:, 1:2], mrg_sb[:, 2:3])
        # preload the exp table set now; overlaps normalize/projection
        nc.scalar.activation(out=actscr[:], in_=eps_sb[:], func=AF.Exp)
        ep = prodpool.tile([2 * C, 3], F32, tag="prod")
        nc.tensor.matmul(ep[:], expand2_sb[:], mrg_sb[:], start=True, stop=True)
        nc.vector.tensor_copy(mrc2_sb[:], ep[:])
        # bq c-fold: adjusted mean m' = m - c*sd so (x-m')*r = xn + c
        nc.vector.tensor_mul(tmpc_sb[:], c2_sb[:], mrc2_sb[:, 2:3])
        nc.vector.tensor_sub(mrc2_sb[:, 0:1], mrc2_sb[:, 0:1], tmpc_sb[:])

        # ---- production: normalize + projections (pull-scheduled) ----
        vt_view = vt_sb[:].rearrange("p (t e) -> p t e", e=C + 1)

        def emit_xh(ch):
            sl = slice(ch * xsz, (ch + 1) * xsz)
            for h in (0, 1):
                hs = slice(h * C, (h + 1) * C)
                nc.vector.tensor_scalar(
                    out=xh2_sb[hs, sl], in0=xb2_sb[hs, sl],
                    scalar1=mrc2_sb[hs, 0:1], scalar2=mrc2_sb[hs, 1:2],
                    op0=OP.subtract, op1=OP.mult,
                )

        def proj_q(j):
            sl = slice(j * PCH, (j + 1) * PCH)
            qp = prodpool.tile([2 * C, PCH], F32, tag="prod", name="qp")
            nc.tensor.matmul(qp[:], wq4_sb[:], xh2_sb[:, sl], start=True, stop=True)
            nc.vector.tensor_copy(q2_sb[:, sl], qp[:])

        def proj_k(j):
            # chunk j = tiles 4j..4j+3 = pairs 2j, 2j+1; pack even tiles into
            # the top k2 half, odd into the bottom (partition-aligned copies)
            sl = slice(j * PCH, (j + 1) * PCH)
            kp = prodpool.tile([2 * C, PCH], F32, tag="prod", name="kp")
            nc.tensor.matmul(kp[:], wk4_sb[:], xh2_sb[:, sl], start=True, stop=True)
            kv = kp[:].rearrange("p (a h c) -> p a h c", a=2, h=2)
            for h in (0, 1):
                hs = slice(h * C, (h + 1) * C)
                dst = k2_sb[hs, 2 * j * NT : (2 * j + 2) * NT]
                nc.vector.tensor_copy(
                    dst.rearrange("p (a c) -> p a c", a=2), kv[hs, :, h, :],
                )

        def proj_vt(b):
            # batch b = tiles 8b..8b+7, one matmul each, single strided copy
            t0 = b * 8
            vp = prodpool.tile([NT, 8 * C], F32, tag="prod", name="vp")
            for t in range(8):
                nc.tensor.matmul(
                    vp[:, t * C : (t + 1) * C],
                    xh2_sb[:, (t0 + t) * NT : (t0 + t + 1) * NT],
                    wv2_sb[:], start=True, stop=True,
                )
            nc.vector.tensor_copy(
                vt_view[:, t0 : t0 + 8, 0:C],
                vp[:].rearrange("p (t e) -> p t e", e=C),
            )

        steps = []
        for c in range(xch):
            steps.append(("xh", c))
            if c == 0:
                for j in range(m_tok // PCH):
                    steps.append(("q", j))
            base_k, base_v = 4 * c, 2 * c
            steps.append(("k", base_k))
            steps.append(("k", base_k + 1))
            steps.append(("vt", base_v))
            steps.append(("k", base_k + 2))
            steps.append(("k", base_k + 3))
            steps.append(("vt", base_v + 1))

        state = {"pairs": 0, "vt": 0}

        def pull(need_pairs=0, need_vt=0):
            while steps and (state["pairs"] < need_pairs or state["vt"] < need_vt):
                kind, arg = steps.pop(0)
                if kind == "xh":
                    emit_xh(arg)
                elif kind == "q":
                    proj_q(arg)
                elif kind == "k":
                    proj_k(arg)
                    state["pairs"] += 2
                else:
                    proj_vt(arg)
                    state["vt"] += 8

        # ---- attention: per 256-query block, row-tiled score pairs ->
        # fp32 PSUM -> exp (fp16) -> col-tiled AV pairs + 4x col-tiled
        # ones-matmul rowsums, accumulated in one shared PSUM bank ----
        def make_tail(av_sb, msl, last):
            def tail():
                recip = mpool.tile([1, MB], F32, tag="recip", name="recip")
                nc.vector.reciprocal(recip[:], av_sb[C : C + 1, :])
                # partition-broadcast recip via a DRAM bounce (SBUF-source
                # DMA cannot replicate across partitions)
                rd = dpool.tile([1, MB], F32, tag="rd", name="rd")
                nc.sync.dma_start(out=rd[:], in_=recip[:])
                rb = mpool.tile([C, MB], F32, tag="rb", name="rb")
                nc.sync.dma_start(out=rb[:], in_=rd[:].to_broadcast([C, MB]))
                t1 = mpool.tile([C, MB], F32, tag="t1", name="t1")
                nc.vector.tensor_mul(t1[:], av_sb[0:C, :], rb[:])
                outt = mpool.tile([C, MB], F32, tag="outt", name="outt")
                nc.vector.scalar_tensor_tensor(
                    out=outt[:], in0=t1[:], scalar=bpc_sb[:], in1=xb2_sb[0:C, msl],
                    op0=OP.add, op1=OP.add,
                )
                if last:
                    h = MB // 2
                    nc.sync.dma_start(
                        out=out_d[:, msl.start : msl.start + h], in_=outt[:, 0:h])
                    nc.sync.dma_start(
                        out=out_d[:, msl.start + h : msl.stop], in_=outt[:, h:])
                else:
                    nc.sync.dma_start(out=out_d[:, msl], in_=outt[:])
            return tail

        deferred = None
        for mb in range(nblk):
            msl = slice(mb * MB, (mb + 1) * MB)
            av = avpool.tile([C + 1, MB], F32, tag="av")
            exmap = {}
            pend = None

            def do_av_rs(g):
                gsz = gsize(g)
                if mb == 0:
                    pull(need_vt=min(g * GSZ + gsz, ntiles))
                ex = exmap[g]
                half = gsz // 2
                first = (g == 0)
                last = (g == ngroups - 1)
                for s in range(gsz):
                    # slot permutation (see scores loop): slot s holds tile
                    # 2*(g*GSZ//2 + s%half) + s//half
                    t = 2 * ((g * GSZ) // 2 + s % half) + s // half
                    nc.tensor.matmul(
                        av[:], vt_view[:, t, :],
                        ex[:, s * MB : (s + 1) * MB],
                        start=(first and s == 0), stop=(last and s == gsz - 1),
                    )

            for g in range(ngroups):
                gsz = gsize(g)
                sp = spool.tile([NT, gsz * MB], F32, tag="s")
                half = gsz // 2
                for i in range(half):
                    p = (g * GSZ) // 2 + i
                    if mb == 0:
                        pull(need_pairs=min(p + 2, npairs))
                    for h in (0, 1):
                        # the two concurrent row-tiled halves MUST write
                        # different PSUM banks (same-bank concurrent PE
                        # writes fault the exec unit -- measured); slot
                        # i + half*h puts them 1.5 banks apart
                        s = i + half * h
                        nc.tensor.matmul(
                            sp[:, s * MB : (s + 1) * MB],
                            k2_sb[h * C : (h + 1) * C, p * NT : (p + 1) * NT],
                            q2_sb[h * C : (h + 1) * C, msl],
                            start=True, stop=True,
                        )
                ex = epool.tile([NT, gsz * MB], F16, tag="e")
                nc.scalar.activation(out=ex[:], in_=sp[:], func=AF.Exp)
                exmap[g] = ex
                if pend is not None:
                    do_av_rs(pend)
                pend = g
                if g == 4 and deferred is not None:
                    deferred()
                    deferred = None
            do_av_rs(pend)

            av_sb = mpool.tile([C + 1, MB], F32, tag="avsb", name="av_sb")
            nc.vector.tensor_copy(av_sb[:], av[:])
            if deferred is not None:
                deferred()
            deferred = make_tail(av_sb, msl, last=(mb == nblk - 1))
        deferred()


def build_program(n_tok=N_FULL, m_tok=M_FULL):
    nc = bacc.Bacc("TRN2", target_bir_lowering=False, debug=False)
    xb_d = nc.dram_tensor("xb", [C, n_tok], F32, kind="ExternalInput")
    wq4_d = nc.dram_tensor("wq4", [2 * C, 2 * C], F16, kind="ExternalInput")
    wk4_d = nc.dram_tensor("wk4", [2 * C, 2 * C], F16, kind="ExternalInput")
    wv2_d = nc.dram_tensor("wv2", [2 * C, C], F16, kind="ExternalInput")
    bpc_d = nc.dram_tensor("bpc", [C, 1], F32, kind="ExternalInput")
    c2_d = nc.dram_tensor("c2", [2 * C, 1], F32, kind="ExternalInput")
    pair_d = nc.dram_tensor("pair", [C, GROUPS], F32, kind="ExternalInput")
    expand2_d = nc.dram_tensor("expand2", [GROUPS, 2 * C], F32, kind="ExternalInput")
    out_d = nc.dram_tensor("out", [C, m_tok], F32, kind="ExternalOutput")
    with tile.TileContext(nc) as tc:
        emit(tc, nc, n_tok, m_tok,
             xb_d.ap(), wq4_d.ap(), wk4_d.ap(), wv2_d.ap(), bpc_d.ap(),
             c2_d.ap(), pair_d.ap(), expand2_d.ap(), out_d.ap())
    nc.compile()
    return nc


def _split16(a):
    hi = a.astype(np.float16)
    lo = (a - hi.astype(np.float32)).astype(np.float16)
    return hi, lo


def prep_weights(gamma, beta, wq, bq, wk, bk, wv, bv, wp, bp, n_tok=N_FULL):
    """Host-side algebraic folds. Returns the shared per-core input dict."""
    f32 = np.float32
    gamma, beta = gamma.astype(f32), beta.astype(f32)
    scale = f32(1.0) / np.sqrt(f32(C)).astype(f32)
    wq_eff = (wq * gamma[None, :]) * scale
    bq_eff = (wq @ beta + bq) * scale
    wk_eff = wk * gamma[None, :]
    wv_eff = wv * gamma[None, :]
    bv_eff = wv @ beta + bv
    wpv_eff = (wp @ wv_eff).astype(f32)

    if np.abs(bq_eff).max() > 0:
        c = np.linalg.lstsq(wq_eff, bq_eff, rcond=None)[0].astype(f32)
    else:
        c = np.zeros(C, f32)
    bp_eff = (bp + wp @ bv_eff - wpv_eff @ c).astype(f32)

    pair = np.zeros((C, GROUPS), f32)
    pair[np.arange(C), np.arange(C) // 2] = f32(1.0) / f32(2 * n_tok)
    expand2 = np.zeros((GROUPS, 2 * C), f32)
    expand2[np.arange(2 * C) % C // 2, np.arange(2 * C)] = 1.0

    def stack16(a):
        hi, lo = _split16(np.ascontiguousarray(a, f32))
        return np.ascontiguousarray(np.concatenate([hi, lo], axis=0))

    def dup_cols(a):  # [128, 64] -> [128, 128]
        return np.ascontiguousarray(np.concatenate([a, a], axis=1))

    return {
        "wq4": dup_cols(stack16(wq_eff.T)),
        "wk4": dup_cols(stack16(wk_eff.T)),
        "wv2": stack16(wpv_eff.T),
        "bpc": bp_eff.reshape(C, 1),
        "c2": np.ascontiguousarray(np.concatenate([c, c]).reshape(2 * C, 1)),
        "pair": pair,
        "expand2": expand2,
    }


_PROGRAM_CACHE = {}


def _get_program(n_tok, m_tok):
    key = (n_tok, m_tok)
    if key not in _PROGRAM_CACHE:
        _PROGRAM_CACHE[key] = build_program(n_tok, m_tok)
    return _PROGRAM_CACHE[key]


def make_in_maps(x, shared):
    """Per-core input maps: batch b = core//4, query chunk qc = core%4."""
    in_maps = []
    for core in range(N_CORES):
        b, qc = core // Q_CHUNKS, core % Q_CHUNKS
        xb = np.ascontiguousarray(x[b].reshape(C, N_FULL), np.float32)
        xb = np.ascontiguousarray(np.roll(xb, -qc * M_FULL, axis=1))
        in_maps.append({"xb": xb, **shared})
    return in_maps


def kernel(x, gamma, beta, wq, bq, wk, bk, wv, bv, wp, bp, **run_kwargs):
    from concourse.bass_utils import run_bass_kernel_spmd

    x = np.asarray(x, np.float32)
    shared = prep_weights(
        np.asarray(gamma), np.asarray(beta), np.asarray(wq), np.asarray(bq),
        np.asarray(wk), np.asarray(bk), np.asarray(wv), np.asarray(bv),
        np.asarray(wp), np.asarray(bp),
    )
    nc = _get_program(N_FULL, M_FULL)
    in_maps = make_in_maps(x, shared)
    res = run_bass_kernel_spmd(nc, in_maps, core_ids=list(range(N_CORES)), **run_kwargs)
    y = np.empty((B_FULL, C, N_FULL), np.float32)
    for core in range(N_CORES):
        b, qc = core // Q_CHUNKS, core % Q_CHUNKS
        y[b, :, qc * M_FULL : (qc + 1) * M_FULL] = res.results[core]["out"]
    out = y.reshape(B_FULL, C, H_FULL, W_FULL, D_FULL)
    if run_kwargs:
        return out, res
    return out


# revision 16
# speedup vs baseline: 1.1159x; 1.0173x over previous
"""Trainium2 Bass kernel for MemoryEfficientAttnBlock3D.

Computes: y = x + conv1x1(attn(conv1x1_{q,k,v}(groupnorm(x))), wp, bp)
for x of shape (2, 64, 32, 32, 8)  (B=2, C=64, N=8192 tokens per batch).

Sharding: 8 cores = 2 batches x 4 query-chunks of 2048 tokens.  Each core
receives its batch's full token volume ROTATED so that its query chunk is
always tokens [0:2048] -- groupnorm statistics and softmax/AV reductions are
permutation-invariant over kv tokens, so all cores run an identical program.

Algebraic folds done on the host:
  - gamma folds into Wq/Wk/Wv columns; the attention scale 1/sqrt(C) into Wq.
  - k's additive constant (Wk@beta + bk) shifts every score of a softmax row
    equally -> dropped exactly (softmax shift invariance).
  - bq enters via a shift of the normalized activations: xh' = xn + c with
    Wq_eff@c = bq_eff.  q picks up bq exactly; k picks up a per-q-row score
    shift (softmax-invariant, cancels between numerator and denominator);
    v picks up Wpv@c which is subtracted from the output bias.  (For this
    problem bq_eff == 0 so c == 0.)
  - the output projection wp folds into the v weights (wp@Wv_eff), so the AV
    matmul accumulates wp@AV directly.
  - v's additive constant and bp are applied as a per-partition scalar in
    the final fused (t + bp) + x DVE op.

Tensor-engine packing (the attention matmuls only use half the 128x128 PE
array naively: K=64 contraction for scores, M=64 output for AV):
  - scores are ROW-TILED 2x: kv tiles are packed in pairs into k2
    [128, n/2] (tile 2p on partitions 0:64, tile 2p+1 on 64:128) and q is
    duplicated into both partition halves; two concurrent K=64 matmuls
    (tile_position (0,0) and (64,0)) produce two score tiles per 256-column
    stream.
  - AV keeps the proven 65-wide form (vt carries a ones column whose
    output row accumulates the softmax denominators); col-tiled partition-
    offset PSUM outputs are not lowered correctly by this stack (measured:
    device fault + CoreSim zero-region corruption), so M=65 stays.
  - projections stack the fp16 hi/lo weight split along the contraction
    (xh duplicated into both halves), halving matmul count vs two
    accumulating passes.

The PE runs at the cold 1.2 GHz p-state for the first ~117us of every
execution (power ramp; activity does NOT lift it early -- measured), so PE
work is minimized to keep the kernel ACT-bound: the ~16.8M softmax exps per
core on the Scalar engine (1 elem/lane/cycle @ 1.2 GHz, dtype-independent)
are the hard floor (~110us).
"""

import numpy as np

import concourse.bass as bass
import concourse.tile as tile
from concourse import bacc, bass_isa, mybir

F32 = mybir.dt.float32
F16 = mybir.dt.float16
AF = mybir.ActivationFunctionType
OP = mybir.AluOpType

C = 64
GROUPS = 32
EPS = 1e-6

B_FULL = 2
H_FULL, W_FULL, D_FULL = 32, 32, 8
N_FULL = H_FULL * W_FULL * D_FULL  # 8192 kv tokens per batch
N_CORES = 8
Q_CHUNKS = 4
M_FULL = N_FULL // Q_CHUNKS  # 2048 q tokens per core

MB = 256        # q-token block
NT = 128        # kv-token tile (matmul M / partition dim)
GSZ = 6         # kv tiles per exp group ([128, 1536] PSUM tile = 3 banks)
STAT_CHUNK = 512
PCH = 512       # projection chunk (tokens)


def emit(tc, nc, n_tok, m_tok, xb_d, wq4_d, wk4_d, wv2_d, bpc_d, c2_d,
         pair_d, expand2_d, out_d):
    ntiles = n_tok // NT          # 64
    npairs = ntiles // 2          # 32
    ngroups = (ntiles + GSZ - 1) // GSZ  # 11
    nch = max(1, n_tok // STAT_CHUNK)
    sch = n_tok // nch
    nkch = n_tok // PCH           # 16 k-projection chunks
    nvb = ntiles // 8             # 8 vt-projection batches
    xch = max(1, n_tok // 2048)   # 4 normalize macro-chunks
    xsz = n_tok // xch
    nblk = m_tok // MB            # 8

    def gsize(g):
        return min(GSZ, ntiles - g * GSZ)

    with (
        tc.tile_pool(name="persist", bufs=1) as persist,
        tc.tile_pool(name="expS", bufs=4) as epool,
        tc.tile_pool(name="mtail", bufs=3) as mpool,
        tc.tile_pool(name="spsum", bufs=2, space="PSUM") as spool,
        tc.tile_pool(name="avpsum", bufs=1, space="PSUM") as avpool,
        tc.tile_pool(name="prodp", bufs=1, space="PSUM") as prodpool,
        tc.tile_pool(name="dram", bufs=2, space="DRAM") as dpool,
    ):
        # ---- persistent SBUF tensors ----
        xb2_sb = persist.tile([2 * C, n_tok], F32)
        xh2_sb = persist.tile([2 * C, n_tok], F16)   # normalized, dup halves
        k2_sb = persist.tile([2 * C, n_tok // 2], F16)  # kv tile pairs
        q2_sb = persist.tile([2 * C, m_tok], F16)    # dup halves
        vt_sb = persist.tile([NT, ntiles * (C + 1)], F16)
        wq4_sb = persist.tile([2 * C, 2 * C], F16)
        wk4_sb = persist.tile([2 * C, 2 * C], F16)
        wv2_sb = persist.tile([2 * C, C], F16)
        bpc_sb = persist.tile([C, 1], F32)
        c2_sb = persist.tile([2 * C, 1], F32)
        pair_sb = persist.tile([C, GROUPS], F32)
        expand2_sb = persist.tile([GROUPS, 2 * C], F32)
        stats_sb = persist.tile([C, 2 * nch], F32)
        scratch_sb = persist.tile([C, sch], F32)
        scratch2_sb = persist.tile([C, sch], F32)
        eps_sb = persist.tile([GROUPS, 1], F32)
        mrg_sb = persist.tile([GROUPS, 3], F32)      # [mean, rstd, sd]
        mrc2_sb = persist.tile([2 * C, 3], F32)
        tmpc_sb = persist.tile([2 * C, 1], F32)
        warm_sb = persist.tile([NT, 512], F16)
        actscr = persist.tile([GROUPS, 1], F32)

        # ---- PE warm-up burst + ACT table preloads at t=0 ----
        nc.vector.memset(warm_sb[:], 0.25)
        nc.vector.memset(eps_sb[:], EPS)
        for _ in range(10):
            warm_ps = prodpool.tile([NT, 512], F32, tag="prod", name="warm_ps")
            nc.tensor.matmul(
                warm_ps[:], warm_sb[:, 0:NT], warm_sb[:], start=True, stop=True,
            )
        # sqrt table set loads in the DMA shadow; Square runs under it
        nc.scalar.activation(out=actscr[:], in_=eps_sb[:], func=AF.Sqrt)

        # x first: it gates the whole stats -> normalize -> project chain
        # (duplicated into both partition halves: engines cannot address
        # different base partitions in one op, so the bottom-half normalize
        # needs its own copy of x)
        for ch in range(nch):
            sl = slice(ch * sch, (ch + 1) * sch)
            nc.sync.dma_start(out=xb2_sb[0:C, sl], in_=xb_d[:, sl])
        for ch in range(xch):
            sl = slice(ch * xsz, (ch + 1) * xsz)
            nc.sync.dma_start(out=xb2_sb[C : 2 * C, sl], in_=xb_d[:, sl])
        nc.sync.dma_start(out=wq4_sb[:], in_=wq4_d[:, :])
        nc.sync.dma_start(out=wk4_sb[:], in_=wk4_d[:, :])
        nc.sync.dma_start(out=wv2_sb[:], in_=wv2_d[:, :])
        nc.sync.dma_start(out=bpc_sb[:], in_=bpc_d[:, :])
        nc.sync.dma_start(out=c2_sb[:], in_=c2_d[:, :])
        nc.sync.dma_start(out=pair_sb[:], in_=pair_d[:, :])
        nc.sync.dma_start(out=expand2_sb[:], in_=expand2_d[:, :])
        # ones column (col C of each 65-wide v^T block) -> AV rowsum
        nc.gpsimd.memset(vt_sb[:], 1.0)

        # ---- per-channel sum / sum-of-squares ----
        for ch in range(nch):
            sl = slice(ch * sch, (ch + 1) * sch)
            nc.vector.tensor_scalar(
                out=scratch2_sb[:], in0=xb2_sb[0:C, sl], scalar1=1.0, scalar2=None,
                op0=OP.mult, op1=OP.add, accum_out=stats_sb[:, ch : ch + 1],
            )
            nc.vector.scalar_tensor_tensor(
                out=scratch_sb[:], in0=xb2_sb[0:C, sl], scalar=1.0,
                in1=xb2_sb[0:C, sl], op0=OP.mult, op1=OP.mult,
                accum_out=stats_sb[:, nch + ch : nch + ch + 1],
            )

        # ---- group statistics ----
        gp = prodpool.tile([GROUPS, 2 * nch], F32, tag="prod")
        nc.tensor.matmul(gp[:], pair_sb[:], stats_sb[:], start=True, stop=True)
        gsum = mpool.tile([GROUPS, 2], F32, tag="gsum")
        nc.vector.tensor_reduce(
            out=gsum[:], in_=gp[:].rearrange("p (s c) -> p s c", s=2),
            axis=mybir.AxisListType.X, op=OP.add,
        )
        msq = mpool.tile([GROUPS, 1], F32, tag="msq")
        nc.vector.tensor_mul(msq[:], gsum[:, 0:1], gsum[:, 0:1])
        nc.vector.tensor_copy(mrg_sb[:, 0:1], gsum[:, 0:1])
        nc.vector.tensor_sub(mrg_sb[:, 2:3], gsum[:, 1:2], msq[:])
        nc.scalar.activation(
            out=mrg_sb[:, 2:3], in_=mrg_sb[:, 2:3], func=AF.Sqrt, bias=eps_sb[:],
        )
        nc.vector.reciprocal(mrg_sb[:, 1:2], mrg_sb[:, 2:3])
        # preload the exp table set now; overlaps normalize/projection
        nc.scalar.activation(out=actscr[:], in_=eps_sb[:], func=AF.Exp)
        ep = prodpool.tile([2 * C, 3], F32, tag="prod")
        nc.tensor.matmul(ep[:], expand2_sb[:], mrg_sb[:], start=True, stop=True)
        nc.vector.tensor_copy(mrc2_sb[:], ep[:])
        # bq c-fold: adjusted mean m' = m - c*sd so (x-m')*r = xn + c
        nc.vector.tensor_mul(tmpc_sb[:], c2_sb[:], mrc2_sb[:, 2:3])
        nc.vector.tensor_sub(mrc2_sb[:, 0:1], mrc2_sb[:, 0:1], tmpc_sb[:])

        # ---- production: normalize + projections (pull-scheduled) ----
        vt_view = vt_sb[:].rearrange("p (t e) -> p t e", e=C + 1)

        def emit_xh(ch):
            sl = slice(ch * xsz, (ch + 1) * xsz)
            for h in (0, 1):
                hs = slice(h * C, (h + 1) * C)
                nc.vector.tensor_scalar(
                    out=xh2_sb[hs, sl], in0=xb2_sb[hs, sl],
                    scalar1=mrc2_sb[hs, 0:1], scalar2=mrc2_sb[hs, 1:2],
                    op0=OP.subtract, op1=OP.mult,
                )

        def proj_q(j):
            sl = slice(j * PCH, (j + 1) * PCH)
            qp = prodpool.tile([2 * C, PCH], F32, tag="prod", name="qp")
            nc.tensor.matmul(qp[:], wq4_sb[:], xh2_sb[:, sl], start=True, stop=True)
            nc.vector.tensor_copy(q2_sb[:, sl], qp[:])

        def proj_k(j):
            # chunk j = tiles 4j..4j+3 = pairs 2j, 2j+1; pack even tiles into
            # the top k2 half, odd into the bottom (partition-aligned copies)
            sl = slice(j * PCH, (j + 1) * PCH)
            kp = prodpool.tile([2 * C, PCH], F32, tag="prod", name="kp")
            nc.tensor.matmul(kp[:], wk4_sb[:], xh2_sb[:, sl], start=True, stop=True)
            kv = kp[:].rearrange("p (a h c) -> p a h c", a=2, h=2)
            for h in (0, 1):
                hs = slice(h * C, (h + 1) * C)
                dst = k2_sb[hs, 2 * j * NT : (2 * j + 2) * NT]
                nc.vector.tensor_copy(
                    dst.rearrange("p (a c) -> p a c", a=2), kv[hs, :, h, :],
                )

        def proj_vt(b):
            # batch b = tiles 8b..8b+7, one matmul each, single strided copy
            t0 = b * 8
            vp = prodpool.tile([NT, 8 * C], F32, tag="prod", name="vp")
            for t in range(8):
                nc.tensor.matmul(
                    vp[:, t * C : (t + 1) * C],
                    xh2_sb[:, (t0 + t) * NT : (t0 + t + 1) * NT],
                    wv2_sb[:], start=True, stop=True,
                )
            nc.vector.tensor_copy(
                vt_view[:, t0 : t0 + 8, 0:C],
                vp[:].rearrange("p (t e) -> p t e", e=C),
            )

        steps = []
        for c in range(xch):
            steps.append(("xh", c))
            if c == 0:
                for j in range(m_tok // PCH):
                    steps.append(("q", j))
            base_k, base_v = 4 * c, 2 * c
            steps.append(("k", base_k))
            steps.append(("k", base_k + 1))
            steps.append(("vt", base_v))
            steps.append(("k", base_k + 2))
            steps.append(("k", base_k + 3))
            steps.append(("vt", base_v + 1))

        state = {"pairs": 0, "vt": 0}

        def pull(need_pairs=0, need_vt=0):
            while steps and (state["pairs"] < need_pairs or state["vt"] < need_vt):
                kind, arg = steps.pop(0)
                if kind == "xh":
                    emit_xh(arg)
                elif kind == "q":
                    proj_q(arg)
                elif kind == "k":
                    proj_k(arg)
                    state["pairs"] += 2
                else:
                    proj_vt(arg)
                    state["vt"] += 8

        # ---- attention: per 256-query block, row-tiled score pairs ->
        # fp32 PSUM -> exp (fp16) -> col-tiled AV pairs + 4x col-tiled
        # ones-matmul rowsums, accumulated in one shared PSUM bank ----
        def make_tail(av_sb, msl, last):
            def tail():
                recip = mpool.tile([1, MB], F32, tag="recip", name="recip")
                nc.vector.reciprocal(recip[:], av_sb[C : C + 1, :])
                # partition-broadcast recip via a DRAM bounce (SBUF-source
                # DMA cannot replicate across partitions)
                rd = dpool.tile([1, MB], F32, tag="rd", name="rd")
                nc.sync.dma_start(out=rd[:], in_=recip[:])
                rb = mpool.tile([C, MB], F32, tag="rb", name="rb")
                nc.sync.dma_start(out=rb[:], in_=rd[:].to_broadcast([C, MB]))
                t1 = mpool.tile([C, MB], F32, tag="t1", name="t1")
                nc.vector.tensor_mul(t1[:], av_sb[0:C, :], rb[:])
                outt = mpool.tile([C, MB], F32, tag="outt", name="outt")
                nc.vector.scalar_tensor_tensor(
                    out=outt[:], in0=t1[:], scalar=bpc_sb[:], in1=xb2_sb[0:C, msl],
                    op0=OP.add, op1=OP.add,
                )
                if last:
                    h = MB // 2
                    nc.sync.dma_start(
                        out=out_d[:, msl.start : msl.start + h], in_=outt[:, 0:h])
                    nc.sync.dma_start(
                        out=out_d[:, msl.start + h : msl.stop], in_=outt[:, h:])
                else:
                    nc.sync.dma_start(out=out_d[:, msl], in_=outt[:])
            return tail

        deferred = None
        deferred_av = None
        for mb in range(nblk):
            msl = slice(mb * MB, (mb + 1) * MB)
            av = avpool.tile([C + 1, MB], F32, tag="av")
            exmap = {}
            pend = None

            def do_av_rs(g, exmap=exmap, av=av, mb=mb):
                gsz = gsize(g)
                if mb == 0:
                    pull(need_vt=min(g * GSZ + gsz, ntiles))
                ex = exmap[g]
                half = gsz // 2
                first = (g == 0)
                last = (g == ngroups - 1)
                for s in range(gsz):
                    # slot permutation (see scores loop): slot s holds tile
                    # 2*(g*GSZ//2 + s%half) + s//half
                    t = 2 * ((g * GSZ) // 2 + s % half) + s // half
                    nc.tensor.matmul(
                        av[:], vt_view[:, t, :],
                        ex[:, s * MB : (s + 1) * MB],
                        start=(first and s == 0), stop=(last and s == gsz - 1),
                    )

            for g in range(ngroups):
                gsz = gsize(g)
                sp = spool.tile([NT, gsz * MB], F32, tag="s")
                half = gsz // 2
                for i in range(half):
                    p = (g * GSZ) // 2 + i
                    if mb == 0:
                        pull(need_pairs=min(p + 2, npairs))
                    for h in (0, 1):
                        # the two concurrent row-tiled halves MUST write
                        # different PSUM banks (same-bank concurrent PE
                        # writes fault the exec unit -- measured); slot
                        # i + half*h puts them 1.5 banks apart
                        s = i + half * h
                        nc.tensor.matmul(
                            sp[:, s * MB : (s + 1) * MB],
                            k2_sb[h * C : (h + 1) * C, p * NT : (p + 1) * NT],
                            q2_sb[h * C : (h + 1) * C, msl],
                            start=True, stop=True,
                        )
                ex = epool.tile([NT, gsz * MB], F16, tag="e")
                nc.scalar.activation(out=ex[:], in_=sp[:], func=AF.Exp)
                exmap[g] = ex
                if g == 1 and deferred_av is not None:
                    # previous block's last AV group + accumulator
                    # evacuation, deferred so this block's first scores
                    # (and exps) aren't stuck behind the PE's AV tail
                    deferred_av()
                    deferred_av = None
                if pend is not None:
                    do_av_rs(pend)
                pend = g
                if g == 4 and deferred is not None:
                    deferred()
                    deferred = None
            last_pend = pend

            def finish_block(do_av_rs_f, av_t, g, msl_, is_last):
                # explicit capture: do_av_rs/av/exmap are rebound per block
                def fin():
                    nonlocal deferred
                    do_av_rs_f(g)
                    av_sb = mpool.tile(
                        [C + 1, MB], F32, tag="avsb", name="av_sb")
                    nc.vector.tensor_copy(av_sb[:], av_t[:])
                    if deferred is not None:
                        deferred()
                    deferred = make_tail(av_sb, msl_, last=is_last)
                return fin

            deferred_av = finish_block(do_av_rs, av, last_pend, msl, mb == nblk - 1)
            if mb == nblk - 1:
                deferred_av()
                deferred_av = None
        deferred()


def build_program(n_tok=N_FULL, m_tok=M_FULL):
    nc = bacc.Bacc("TRN2", target_bir_lowering=False, debug=False)
    xb_d = nc.dram_tensor("xb", [C, n_tok], F32, kind="ExternalInput")
    wq4_d = nc.dram_tensor("wq4", [2 * C, 2 * C], F16, kind="ExternalInput")
    wk4_d = nc.dram_tensor("wk4", [2 * C, 2 * C], F16, kind="ExternalInput")
    wv2_d = nc.dram_tensor("wv2", [2 * C, C], F16, kind="ExternalInput")
    bpc_d = nc.dram_tensor("bpc", [C, 1], F32, kind="ExternalInput")
    c2_d = nc.dram_tensor("c2", [2 * C, 1], F32, kind="ExternalInput")
    pair_d = nc.dram_tensor("pair", [C, GROUPS], F32, kind="ExternalInput")
    expand2_d = nc.dram_tensor("expand2", [GROUPS, 2 * C], F32, kind="ExternalInput")
    out_d = nc.dram_tensor("out", [C, m_tok], F32, kind="ExternalOutput")
    with tile.TileContext(nc) as tc:
        emit(tc, nc, n_tok, m_tok,
             xb_d.ap(), wq4_d.ap(), wk4_d.ap(), wv2_d.ap(), bpc_d.ap(),
             c2_d.ap(), pair_d.ap(), expand2_d.ap(), out_d.ap())
    nc.compile()
    return nc


def _split16(a):
    hi = a.astype(np.float16)
    lo = (a - hi.astype(np.float32)).astype(np.float16)
    return hi, lo


def prep_weights(gamma, beta, wq, bq, wk, bk, wv, bv, wp, bp, n_tok=N_FULL):
    """Host-side algebraic folds. Returns the shared per-core input dict."""
    f32 = np.float32
    gamma, beta = gamma.astype(f32), beta.astype(f32)
    scale = f32(1.0) / np.sqrt(f32(C)).astype(f32)
    wq_eff = (wq * gamma[None, :]) * scale
    bq_eff = (wq @ beta + bq) * scale
    wk_eff = wk * gamma[None, :]
    wv_eff = wv * gamma[None, :]
    bv_eff = wv @ beta + bv
    wpv_eff = (wp @ wv_eff).astype(f32)

    if np.abs(bq_eff).max() > 0:
        c = np.linalg.lstsq(wq_eff, bq_eff, rcond=None)[0].astype(f32)
    else:
        c = np.zeros(C, f32)
    bp_eff = (bp + wp @ bv_eff - wpv_eff @ c).astype(f32)

    pair = np.zeros((C, GROUPS), f32)
    pair[np.arange(C), np.arange(C) // 2] = f32(1.0) / f32(2 * n_tok)
    expand2 = np.zeros((GROUPS, 2 * C), f32)
    expand2[np.arange(2 * C) % C // 2, np.arange(2 * C)] = 1.0

    def stack16(a):
        hi, lo = _split16(np.ascontiguousarray(a, f32))
        return np.ascontiguousarray(np.concatenate([hi, lo], axis=0))

    def dup_cols(a):  # [128, 64] -> [128, 128]
        return np.ascontiguousarray(np.concatenate([a, a], axis=1))

    return {
        "wq4": dup_cols(stack16(wq_eff.T)),
        "wk4": dup_cols(stack16(wk_eff.T)),
        "wv2": stack16(wpv_eff.T),
        "bpc": bp_eff.reshape(C, 1),
        "c2": np.ascontiguousarray(np.concatenate([c, c]).reshape(2 * C, 1)),
        "pair": pair,
        "expand2": expand2,
    }


_PROGRAM_CACHE = {}


def _get_program(n_tok, m_tok):
    key = (n_tok, m_tok)
    if key not in _PROGRAM_CACHE:
        _PROGRAM_CACHE[key] = build_program(n_tok, m_tok)
    return _PROGRAM_CACHE[key]


def make_in_maps(x, shared):
    """Per-core input maps: batch b = core//4, query chunk qc = core%4."""
    in_maps = []
    for core in range(N_CORES):
        b, qc = core // Q_CHUNKS, core % Q_CHUNKS
        xb = np.ascontiguousarray(x[b].reshape(C, N_FULL), np.float32)
        xb = np.ascontiguousarray(np.roll(xb, -qc * M_FULL, axis=1))
        in_maps.append({"xb": xb, **shared})
    return in_maps


def kernel(x, gamma, beta, wq, bq, wk, bk, wv, bv, wp, bp, **run_kwargs):
    from concourse.bass_utils import run_bass_kernel_spmd

    x = np.asarray(x, np.float32)
    shared = prep_weights(
        np.asarray(gamma), np.asarray(beta), np.asarray(wq), np.asarray(bq),
        np.asarray(wk), np.asarray(bk), np.asarray(wv), np.asarray(bv),
        np.asarray(wp), np.asarray(bp),
    )
    nc = _get_program(N_FULL, M_FULL)
    in_maps = make_in_maps(x, shared)
    res = run_bass_kernel_spmd(nc, in_maps, core_ids=list(range(N_CORES)), **run_kwargs)
    y = np.empty((B_FULL, C, N_FULL), np.float32)
    for core in range(N_CORES):
        b, qc = core // Q_CHUNKS, core % Q_CHUNKS
        y[b, :, qc * M_FULL : (qc + 1) * M_FULL] = res.results[core]["out"]
    out = y.reshape(B_FULL, C, H_FULL, W_FULL, D_FULL)
    if run_kwargs:
        return out, res
    return out
